# revision 19
# baseline (speedup 1.0000x reference)
"""Trainium2 Bass kernel for nn_ArrivalTime (sparse attention over 24 timeslots).

Math refactoring (exact, up to fp reassociation):
  query = [user_pref[user], timeslot[hour]] has only 64 distinct user rows and
  24 distinct time rows, so
    scores[n,h,t] = US[b(n), h, t] + TS[hour[n], h, t]
  with tiny host-precomputed tables
    US = (user_q @ k^T) * scale   [64, H*T]   (user_q folds bq)
    TS = (time_q @ k^T) * scale   [24, H*T]
  Masking adds -1e9 where hour_mask==1.  Softmax per head over t (24).
  Output: out[n,:] = attn[n,:] @ vproj + bu, vproj[(h,t),d] = v[h,t,:]@Wu[d,h*HD:]^T.

Device pipeline (per core, transposed layout: tokens on the free dim), one
iteration per batch row (512 tokens):
  PE : ps_s = table^T @ stream      (one-hot hour + mask rows, bf16)
  ACT: p = exp(ps_s + US_b bias)    (row 96 = exp(0) = 1 -> carries bu)
  PE : ps_z = seg^T @ p             (per-head sums)
  ACT: lnz = ln(ps_z); r = exp(-lnz)     (1/Z without the slow DVE reciprocal)
  PE : ps_r = segT^T @ r            (broadcast 1/Z over each head's 24 rows)
  DVE: p[:96] *= ps_r               (normalize)
  PE : ps_o{h} = vproj[:,h]^T @ p   (two 128-col halves; vproj row 96 = bu)
  DVE: copy psum -> sbuf; SYNC triggers the output DMAs.

Sharding: data-parallel over batch, 8 batch rows (= 8 x 512 tokens) per core.
Raw bass (no Tile): this toolchain's walrus allows at most one attached sem
wait per instruction, so all waits are standalone wait_ge ops with manually
counted thresholds.
"""

import os
import numpy as np

B, S, D, H, HD, T = 64, 512, 256, 4, 64, 24
NCORES = 8
BPC = B // NCORES  # batch rows per core
HT = H * T  # 96
K1 = 2 * T  # 48 stream rows: one-hot hour + mask
MASK_NEG = -1.0e9

# packed bf16 constant tensor [HT+1, CWB]: columns
C_VP = 0                    # vproj_ext [97, 256]
C_SEG2 = C_VP + D           # seg2 [97, 96]: 1 where head(k)==head(m)
C_TAB = C_SEG2 + HT         # table (valid rows :K1) [48, 97]
CWB = C_TAB + (HT + 1)


def _host_tables(timeslot_embedded, user, hour, hour_mask, user_pref,
                 Wq, bq, Wk, bk, Wv, bv, Wu, bu):
    import ml_dtypes
    f32 = np.float32
    bf16 = ml_dtypes.bfloat16
    ts_e = np.asarray(timeslot_embedded, f32)          # [T, D]
    user = np.asarray(user).astype(np.int64)           # [B]
    hour = np.asarray(hour).astype(np.int64)           # [B, S]
    hour_mask = np.asarray(hour_mask)                  # [B, S, T]
    Wq = np.asarray(Wq, f32); bq = np.asarray(bq, f32)
    Wk = np.asarray(Wk, f32); bk = np.asarray(bk, f32)
    Wv = np.asarray(Wv, f32); bv = np.asarray(bv, f32)
    Wu = np.asarray(Wu, f32); bu = np.asarray(bu, f32)

    Wq_u, Wq_t = Wq[:, :, :D], Wq[:, :, D:]
    k_ = np.einsum('td,hkd->htk', ts_e, Wk) + bk[:, None, :]   # [H,T,HD]
    v_ = np.einsum('td,hkd->htk', ts_e, Wv) + bv[:, None, :]
    time_q = np.einsum('td,hkd->thk', ts_e, Wq_t)              # [T,H,HD]
    upref = np.asarray(user_pref, f32)[user]                   # [B,D]
    user_q = np.einsum('bd,hkd->bhk', upref, Wq_u) + bq[None]  # [B,H,HD]
    scale = f32(1.0 / np.sqrt(HD))
    TS = (np.einsum('thk,hsk->ths', time_q, k_) * scale).reshape(T, HT)
    US = (np.einsum('bhk,hsk->bhs', user_q, k_) * scale).reshape(B, HT)
    vproj = np.einsum('htk,dhk->htd', v_, Wu.reshape(D, H, HD)).reshape(HT, D)

    # table [K1, HT+1]: rows 0..23 TS, rows 24..47 mask additive; col 96 = 0
    table = np.zeros((K1, HT + 1), f32)
    table[:T, :HT] = TS
    table[T:, :HT] = np.tile(np.eye(T, dtype=f32), (1, H)) * f32(MASK_NEG)

    seg2 = np.kron(np.eye(H, dtype=f32), np.ones((T, T), f32))  # [HT, HT]

    cb = np.zeros((HT + 1, CWB), f32)
    cb[:HT, C_VP:C_VP + D] = vproj
    cb[HT, C_VP:C_VP + D] = bu
    cb[:HT, C_SEG2:C_SEG2 + HT] = seg2
    cb[:K1, C_TAB:C_TAB + HT + 1] = table
    const_bf = cb.astype(bf16)

    us_all = np.zeros((HT + 1, B), f32)
    us_all[:HT, :] = US.T
    us_cols = [np.ascontiguousarray(us_all[:, c * BPC:(c + 1) * BPC])
               for c in range(NCORES)]

    # per-core streams [BPC, K1, S] bf16: one-hot(hour) rows + mask^T rows
    eyeT = np.eye(T, dtype=f32)
    streams = []
    for c in range(NCORES):
        hb = hour[c * BPC:(c + 1) * BPC]                       # [BPC, S]
        mb = hour_mask[c * BPC:(c + 1) * BPC]                  # [BPC, S, T]
        st = np.empty((BPC, K1, S), f32)
        st[:, :T, :] = eyeT[hb].transpose(0, 2, 1)
        st[:, T:, :] = mb.astype(f32).transpose(0, 2, 1)
        streams.append(st.astype(bf16))
    return const_bf, us_cols, streams


def _build_program():
    import concourse.bass as bass
    import concourse.mybir as mybir
    from contextlib import ExitStack

    f32 = mybir.dt.float32
    bf16 = mybir.dt.bfloat16
    nc = bass.Bass("TRN2")
    stream_d = nc.declare_dram_parameter("stream", [BPC, K1, S], bf16,
                                         isOutput=False)
    const_d = nc.declare_dram_parameter("const", [HT + 1, CWB], bf16,
                                        isOutput=False)
    us_d = nc.declare_dram_parameter("usb", [HT + 1, BPC], f32, isOutput=False)
    out_d = nc.declare_dram_parameter("out", [BPC, D, S], f32, isOutput=True)

    Exp = mybir.ActivationFunctionType.Exp
    Ln = mybir.ActivationFunctionType.Ln

    # Deep software pipeline; 4 matmuls per iteration:
    #   A (scores), hs2 (head-sum REPLICATED via seg2 -> [96,S]), mm2 x2.
    # ln/expneg then produce 1/Z already broadcast ([96,S] costs the same as
    # [4,S] on ACT: free-dim bound), and the DVE mul runs bf16-SBUF at 2x.
    # PE block j emits: mm2ab_{j-2} | hs2_j | A_{j+1}.
    # ACT ticks: exp1_i=3i+1 ln=3i+2 expneg=3i+3.
    # DVE ticks: mul_i=2i+1 ocopy_i=2i+2.  PE ticks recorded at emission.
    with ExitStack() as ctx:
        ec = ctx.enter_context
        const_sb = ec(nc.sbuf_tensor("const_sb", [HT + 1, CWB], bf16))
        us_sb = ec(nc.sbuf_tensor("us_sb", [HT + 1, BPC], f32))
        sts = [ec(nc.sbuf_tensor(f"st{j}", [K1, S], bf16)) for j in range(BPC)]
        ps = [ec(nc.sbuf_tensor(f"p{j}", [HT + 1, S], bf16)) for j in range(5)]
        lnz_sb = ec(nc.sbuf_tensor("lnz_sb", [HT, S], f32))
        r_sbs = [ec(nc.sbuf_tensor(f"r_sb{j}", [HT, S], bf16)) for j in range(3)]
        ots = [ec(nc.sbuf_tensor(f"ot{j}", [128, 2 * S], f32))
               for j in range(2)]
        ps_ss = [ec(nc.psum_tensor(f"ps_s{j}", [HT + 1, S], f32))
                 for j in range(2)]
        ps_zs = [ec(nc.psum_tensor(f"ps_z{j}", [HT, S], f32)) for j in range(2)]
        ps_os = [ec(nc.psum_tensor(f"ps_o{j}", [128, 2 * S], f32))
                 for j in range(2)]
        c_sem = ec(nc.semaphore("c_sem"))
        u_sem = ec(nc.semaphore("u_sem"))
        st_sems = [ec(nc.semaphore(f"st_sem{j}")) for j in range(BPC)]
        pe_sem = ec(nc.semaphore("pe_sem"))
        act_sem = ec(nc.semaphore("act_sem"))
        dve_sem = ec(nc.semaphore("dve_sem"))
        ot_sems = [ec(nc.semaphore(f"ot_sem{j}")) for j in range(2)]
        block = ec(nc.Block(no_gpsimd_drain=True))

        vproj = const_sb[:, C_VP:C_VP + D]
        seg2 = const_sb[:, C_SEG2:C_SEG2 + HT]
        table = const_sb[:K1, C_TAB:C_TAB + HT + 1]

        pe_tick = {}
        pe_cnt = [0]

        @block.tensor
        def _(tensor):
            def mm(key, out, lhsT, rhs):
                tensor.matmul(out, lhsT, rhs,
                              start=True, stop=True).then_inc(pe_sem, 1)
                pe_cnt[0] += 1
                pe_tick[key] = pe_cnt[0]

            tensor.wait_ge(c_sem, 16)
            tensor.wait_ge(st_sems[0], 16)
            mm(('A', 0), ps_ss[0][:], table, sts[0][:])
            for j in range(BPC + 2):
                if 0 <= j - 2 < BPC:            # mm2_{j-2}
                    i = j - 2
                    tensor.wait_ge(dve_sem, 2 * i + 1)   # mul_i done
                    mm(('m2a', i), ps_os[i % 2][:, 0:S],
                       vproj[:, 0:128], ps[i % 5][:])
                    mm(('m2b', i), ps_os[i % 2][:, S:2 * S],
                       vproj[:, 128:256], ps[i % 5][:])
                if j < BPC:                     # hs2_j
                    tensor.wait_ge(act_sem, 3 * j + 1)   # exp1_j done
                    mm(('hs', j), ps_zs[j % 2][:], seg2, ps[j % 5][:])
                if j + 1 < BPC:                 # A_{j+1}
                    i = j + 1
                    tensor.wait_ge(st_sems[i], 16)
                    mm(('A', i), ps_ss[i % 2][:], table, sts[i][:])

        @block.scalar
        def _(scalar):
            for i in range(2, BPC, 2):
                scalar.dma_start(sts[i][:],
                                 stream_d[i]).then_inc(st_sems[i], 16)
            scalar.wait_ge(u_sem, 16)
            for i in range(BPC):
                scalar.wait_ge(pe_sem, pe_tick[('A', i)])
                scalar.activation(ps[i % 5][:], ps_ss[i % 2][:], Exp,
                                  bias=us_sb[:, i:i + 1],
                                  scale=1.0).then_inc(act_sem, 1)
                scalar.wait_ge(pe_sem, pe_tick[('hs', i)])
                scalar.activation(lnz_sb[:],
                                  ps_zs[i % 2][:], Ln).then_inc(act_sem, 1)
                scalar.activation(r_sbs[i % 3][:], lnz_sb[:], Exp,
                                  scale=-1.0).then_inc(act_sem, 1)

        @block.vector
        def _(vector):
            for i in range(BPC):
                vector.wait_ge(pe_sem, pe_tick[('hs', i)])
                vector.wait_ge(act_sem, 3 * i + 3)    # expneg_i done
                vector.tensor_mul(ps[i % 5][:HT, :], ps[i % 5][:HT, :],
                                  r_sbs[i % 3][:]).then_inc(dve_sem, 1)
                vector.wait_ge(pe_sem, pe_tick[('m2b', i)])
                if i >= 2:
                    vector.wait_ge(ot_sems[i % 2], 16 * (i // 2))
                vector.tensor_copy(ots[i % 2][:],
                                   ps_os[i % 2][:]).then_inc(dve_sem, 1)

        @block.sync
        def _(sync):
            for i in range(1, BPC, 2):
                sync.dma_start(sts[i][:], stream_d[i]).then_inc(st_sems[i], 16)
            for i in range(BPC):
                sync.wait_ge(dve_sem, 2 * i + 2)      # ocopy_i done
                dest = out_d[i, :, :].rearrange("(h p) s -> p h s", h=2)
                src = ots[i % 2][:, :].rearrange("p (h s) -> p h s", h=2)
                sync.dma_start(dest, src).then_inc(ot_sems[i % 2], 16)
            for bb in range(2):
                cnt = len([i for i in range(BPC) if i % 2 == bb])
                sync.wait_ge(ot_sems[bb], 16 * cnt)

        @block.gpsimd
        def _(gpsimd):
            gpsimd.dma_start(const_sb[:], const_d[:]).then_inc(c_sem, 16)
            gpsimd.dma_start(sts[0][:], stream_d[0]).then_inc(st_sems[0], 16)
            gpsimd.dma_start(us_sb[:], us_d[:]).then_inc(u_sem, 16)
    return nc


def _run(inputs, trace=False):
    import sys
    if "/opt/trn_rl_repo" not in sys.path:
        sys.path.insert(0, "/opt/trn_rl_repo")
    from concourse.bass_utils import run_bass_kernel_spmd

    const_bf, us_cols, streams = _host_tables(**inputs)
    nc = _build_program()
    in_maps = [
        {"stream": streams[c], "const": const_bf, "usb": us_cols[c]}
        for c in range(NCORES)
    ]
    res = run_bass_kernel_spmd(nc, in_maps, core_ids=list(range(NCORES)),
                               trace=trace)
    out_full = np.empty((B, S, D), np.float32)
    for c in range(NCORES):
        oc = res.results[c]["out"]  # [BPC, D, S]
        out_full[c * BPC:(c + 1) * BPC] = oc.transpose(0, 2, 1)
    return out_full, res


def kernel(**inputs):
    trace = bool(int(os.environ.get("BASS_KERNEL_TRACE", "0")))
    out, _ = _run(inputs, trace=trace)
    return out


def kernel_profiled(**inputs):
    out, res = _run(inputs, trace=True)
    return out, res


# revision 20
# speedup vs baseline: 1.0265x; 1.0265x over previous
"""Trainium2 Bass kernel for nn_ArrivalTime (sparse attention over 24 timeslots).

Math refactoring (exact, up to fp reassociation):
  query = [user_pref[user], timeslot[hour]] has only 64 distinct user rows and
  24 distinct time rows, so
    scores[n,h,t] = US[b(n), h, t] + TS[hour[n], h, t]
  with tiny host-precomputed tables
    US = (user_q @ k^T) * scale   [64, H*T]   (user_q folds bq)
    TS = (time_q @ k^T) * scale   [24, H*T]
  Masking adds -1e9 where hour_mask==1.  Softmax per head over t (24).
  Output: out[n,:] = attn[n,:] @ vproj + bu, vproj[(h,t),d] = v[h,t,:]@Wu[d,h*HD:]^T.

Device pipeline (per core, transposed layout: tokens on the free dim), one
iteration per batch row (512 tokens):
  PE : ps_s = table^T @ stream      (one-hot hour + mask rows, bf16)
  ACT: p = exp(ps_s + US_b bias)    (row 96 = exp(0) = 1 -> carries bu)
  PE : ps_z = seg^T @ p             (per-head sums)
  ACT: lnz = ln(ps_z); r = exp(-lnz)     (1/Z without the slow DVE reciprocal)
  PE : ps_r = segT^T @ r            (broadcast 1/Z over each head's 24 rows)
  DVE: p[:96] *= ps_r               (normalize)
  PE : ps_o{h} = vproj[:,h]^T @ p   (two 128-col halves; vproj row 96 = bu)
  DVE: copy psum -> sbuf; SYNC triggers the output DMAs.

Sharding: data-parallel over batch, 8 batch rows (= 8 x 512 tokens) per core.
Raw bass (no Tile): this toolchain's walrus allows at most one attached sem
wait per instruction, so all waits are standalone wait_ge ops with manually
counted thresholds.
"""

import os
import numpy as np

B, S, D, H, HD, T = 64, 512, 256, 4, 64, 24
NCORES = 8
BPC = B // NCORES  # batch rows per core
HT = H * T  # 96
K1 = 2 * T  # 48 stream rows: one-hot hour + mask
MASK_NEG = -1.0e9

# packed bf16 constant tensor [HT+1, CWB]: columns
C_VP = 0                    # vproj_ext [97, 256]
C_SEG2 = C_VP + D           # seg2 [97, 96]: 1 where head(k)==head(m)
C_TAB = C_SEG2 + HT         # table (valid rows :K1) [48, 97]
CWB = C_TAB + (HT + 1)


def _host_tables(timeslot_embedded, user, hour, hour_mask, user_pref,
                 Wq, bq, Wk, bk, Wv, bv, Wu, bu):
    import ml_dtypes
    f32 = np.float32
    bf16 = ml_dtypes.bfloat16
    ts_e = np.asarray(timeslot_embedded, f32)          # [T, D]
    user = np.asarray(user).astype(np.int64)           # [B]
    hour = np.asarray(hour).astype(np.int64)           # [B, S]
    hour_mask = np.asarray(hour_mask)                  # [B, S, T]
    Wq = np.asarray(Wq, f32); bq = np.asarray(bq, f32)
    Wk = np.asarray(Wk, f32); bk = np.asarray(bk, f32)
    Wv = np.asarray(Wv, f32); bv = np.asarray(bv, f32)
    Wu = np.asarray(Wu, f32); bu = np.asarray(bu, f32)

    Wq_u, Wq_t = Wq[:, :, :D], Wq[:, :, D:]
    k_ = np.einsum('td,hkd->htk', ts_e, Wk) + bk[:, None, :]   # [H,T,HD]
    v_ = np.einsum('td,hkd->htk', ts_e, Wv) + bv[:, None, :]
    time_q = np.einsum('td,hkd->thk', ts_e, Wq_t)              # [T,H,HD]
    upref = np.asarray(user_pref, f32)[user]                   # [B,D]
    user_q = np.einsum('bd,hkd->bhk', upref, Wq_u) + bq[None]  # [B,H,HD]
    scale = f32(1.0 / np.sqrt(HD))
    TS = (np.einsum('thk,hsk->ths', time_q, k_) * scale).reshape(T, HT)
    US = (np.einsum('bhk,hsk->bhs', user_q, k_) * scale).reshape(B, HT)
    vproj = np.einsum('htk,dhk->htd', v_, Wu.reshape(D, H, HD)).reshape(HT, D)

    # table [K1, HT+1]: rows 0..23 TS, rows 24..47 mask additive; col 96 = 0
    table = np.zeros((K1, HT + 1), f32)
    table[:T, :HT] = TS
    table[T:, :HT] = np.tile(np.eye(T, dtype=f32), (1, H)) * f32(MASK_NEG)

    seg2 = np.kron(np.eye(H, dtype=f32), np.ones((T, T), f32))  # [HT, HT]

    cb = np.zeros((HT + 1, CWB), f32)
    cb[:HT, C_VP:C_VP + D] = vproj
    cb[HT, C_VP:C_VP + D] = bu
    cb[:HT, C_SEG2:C_SEG2 + HT] = seg2
    cb[:K1, C_TAB:C_TAB + HT + 1] = table
    const_bf = cb.astype(bf16)

    us_all = np.zeros((HT + 1, B), f32)
    us_all[:HT, :] = US.T
    us_cols = [np.ascontiguousarray(us_all[:, c * BPC:(c + 1) * BPC])
               for c in range(NCORES)]

    # per-core streams [BPC, K1, S] bf16: one-hot(hour) rows + mask^T rows
    eyeT = np.eye(T, dtype=f32)
    streams = []
    for c in range(NCORES):
        hb = hour[c * BPC:(c + 1) * BPC]                       # [BPC, S]
        mb = hour_mask[c * BPC:(c + 1) * BPC]                  # [BPC, S, T]
        st = np.empty((BPC, K1, S), f32)
        st[:, :T, :] = eyeT[hb].transpose(0, 2, 1)
        st[:, T:, :] = mb.astype(f32).transpose(0, 2, 1)
        streams.append(st.astype(bf16))
    return const_bf, us_cols, streams


def _build_program():
    import concourse.bass as bass
    import concourse.mybir as mybir
    from contextlib import ExitStack

    f32 = mybir.dt.float32
    bf16 = mybir.dt.bfloat16
    nc = bass.Bass("TRN2")
    stream_d = nc.declare_dram_parameter("stream", [BPC, K1, S], bf16,
                                         isOutput=False)
    const_d = nc.declare_dram_parameter("const", [HT + 1, CWB], bf16,
                                        isOutput=False)
    us_d = nc.declare_dram_parameter("usb", [HT + 1, BPC], f32, isOutput=False)
    out_d = nc.declare_dram_parameter("out", [BPC, D, S], f32, isOutput=True)

    Exp = mybir.ActivationFunctionType.Exp
    Ln = mybir.ActivationFunctionType.Ln

    # Deep software pipeline; 4 matmuls per iteration:
    #   A (scores), hs2 (head-sum REPLICATED via seg2 -> [96,S]), mm2 x2.
    # ln/expneg then produce 1/Z already broadcast ([96,S] costs the same as
    # [4,S] on ACT: free-dim bound), and the DVE mul runs bf16-SBUF at 2x.
    # PE block j emits: mm2ab_{j-2} | hs2_j | A_{j+1}.
    # ACT ticks: exp1_i=3i+1 ln=3i+2 expneg=3i+3.
    # DVE ticks: mul_i=2i+1 ocopy_i=2i+2.  PE ticks recorded at emission.
    with ExitStack() as ctx:
        ec = ctx.enter_context
        const_sb = ec(nc.sbuf_tensor("const_sb", [HT + 1, CWB], bf16))
        us_sb = ec(nc.sbuf_tensor("us_sb", [HT + 1, BPC], f32))
        sts = [ec(nc.sbuf_tensor(f"st{j}", [K1, S], bf16)) for j in range(BPC)]
        ps = [ec(nc.sbuf_tensor(f"p{j}", [HT + 1, S], bf16)) for j in range(5)]
        lnz_sb = ec(nc.sbuf_tensor("lnz_sb", [HT, S], f32))
        r_sbs = [ec(nc.sbuf_tensor(f"r_sb{j}", [HT, S], bf16)) for j in range(3)]
        ots = [ec(nc.sbuf_tensor(f"ot{j}", [128, 2 * S], f32))
               for j in range(2)]
        ps_ss = [ec(nc.psum_tensor(f"ps_s{j}", [HT + 1, S], f32))
                 for j in range(2)]
        ps_zs = [ec(nc.psum_tensor(f"ps_z{j}", [HT, S], f32)) for j in range(2)]
        ps_os = [ec(nc.psum_tensor(f"ps_o{j}", [128, 2 * S], f32))
                 for j in range(2)]
        c_sem = ec(nc.semaphore("c_sem"))
        u_sem = ec(nc.semaphore("u_sem"))
        st_sem = ec(nc.semaphore("st_sem"))
        pe_sem = ec(nc.semaphore("pe_sem"))
        act_sem = ec(nc.semaphore("act_sem"))
        dve_sem = ec(nc.semaphore("dve_sem"))
        ot_sems = [ec(nc.semaphore(f"ot_sem{j}")) for j in range(2)]
        block = ec(nc.Block(no_gpsimd_drain=True))

        vproj = const_sb[:, C_VP:C_VP + D]
        seg2 = const_sb[:, C_SEG2:C_SEG2 + HT]
        table = const_sb[:K1, C_TAB:C_TAB + HT + 1]

        pe_tick = {}
        pe_cnt = [0]

        @block.tensor
        def _(tensor):
            def mm(key, out, lhsT, rhs):
                tensor.matmul(out, lhsT, rhs,
                              start=True, stop=True).then_inc(pe_sem, 1)
                pe_cnt[0] += 1
                pe_tick[key] = pe_cnt[0]

            tensor.wait_ge(c_sem, 16)
            tensor.wait_ge(st_sem, 16)
            mm(('A', 0), ps_ss[0][:], table, sts[0][:])
            for j in range(BPC + 2):
                if 0 <= j - 2 < BPC:            # mm2_{j-2}
                    i = j - 2
                    tensor.wait_ge(dve_sem, 2 * i + 1)   # mul_i done
                    mm(('m2a', i), ps_os[i % 2][:, 0:S],
                       vproj[:, 0:128], ps[i % 5][:])
                    mm(('m2b', i), ps_os[i % 2][:, S:2 * S],
                       vproj[:, 128:256], ps[i % 5][:])
                if j < BPC:                     # hs2_j
                    tensor.wait_ge(act_sem, 3 * j + 1)   # exp1_j done
                    mm(('hs', j), ps_zs[j % 2][:], seg2, ps[j % 5][:])
                if j + 1 < BPC:                 # A_{j+1}
                    i = j + 1
                    tensor.wait_ge(st_sem, 16 * (i + 1))
                    mm(('A', i), ps_ss[i % 2][:], table, sts[i][:])

        @block.scalar
        def _(scalar):
            scalar.wait_ge(u_sem, 16)
            for i in range(BPC):
                scalar.wait_ge(pe_sem, pe_tick[('A', i)])
                scalar.activation(ps[i % 5][:], ps_ss[i % 2][:], Exp,
                                  bias=us_sb[:, i:i + 1],
                                  scale=1.0).then_inc(act_sem, 1)
                scalar.wait_ge(pe_sem, pe_tick[('hs', i)])
                scalar.activation(lnz_sb[:],
                                  ps_zs[i % 2][:], Ln).then_inc(act_sem, 1)
                scalar.activation(r_sbs[i % 3][:], lnz_sb[:], Exp,
                                  scale=-1.0).then_inc(act_sem, 1)

        @block.vector
        def _(vector):
            for i in range(BPC):
                vector.wait_ge(pe_sem, pe_tick[('hs', i)])
                vector.wait_ge(act_sem, 3 * i + 3)    # expneg_i done
                vector.tensor_mul(ps[i % 5][:HT, :], ps[i % 5][:HT, :],
                                  r_sbs[i % 3][:]).then_inc(dve_sem, 1)
                vector.wait_ge(pe_sem, pe_tick[('m2b', i)])
                if i >= 2:
                    vector.wait_ge(ot_sems[i % 2], 16 * (i // 2))
                vector.tensor_copy(ots[i % 2][:],
                                   ps_os[i % 2][:]).then_inc(dve_sem, 1)

        @block.sync
        def _(sync):
            sync.dma_start(const_sb[:], const_d[:]).then_inc(c_sem, 16)
            for i in range(0, BPC):
                sync.dma_start(sts[i][:], stream_d[i]).then_inc(st_sem, 16)
            for i in range(BPC):
                sync.wait_ge(dve_sem, 2 * i + 2)      # ocopy_i done
                dest = out_d[i, :, :].rearrange("(h p) s -> p h s", h=2)
                src = ots[i % 2][:, :].rearrange("p (h s) -> p h s", h=2)
                sync.dma_start(dest, src).then_inc(ot_sems[i % 2], 16)
            for bb in range(2):
                cnt = len([i for i in range(BPC) if i % 2 == bb])
                sync.wait_ge(ot_sems[bb], 16 * cnt)

        @block.gpsimd
        def _(gpsimd):
            gpsimd.dma_start(us_sb[:], us_d[:]).then_inc(u_sem, 16)
    return nc


def _run(inputs, trace=False):
    import sys
    if "/opt/trn_rl_repo" not in sys.path:
        sys.path.insert(0, "/opt/trn_rl_repo")
    from concourse.bass_utils import run_bass_kernel_spmd

    const_bf, us_cols, streams = _host_tables(**inputs)
    nc = _build_program()
    in_maps = [
        {"stream": streams[c], "const": const_bf, "usb": us_cols[c]}
        for c in range(NCORES)
    ]
    res = run_bass_kernel_spmd(nc, in_maps, core_ids=list(range(NCORES)),
                               trace=trace)
    out_full = np.empty((B, S, D), np.float32)
    for c in range(NCORES):
        oc = res.results[c]["out"]  # [BPC, D, S]
        out_full[c * BPC:(c + 1) * BPC] = oc.transpose(0, 2, 1)
    return out_full, res


def kernel(**inputs):
    trace = bool(int(os.environ.get("BASS_KERNEL_TRACE", "0")))
    out, _ = _run(inputs, trace=trace)
    return out


def kernel_profiled(**inputs):
    out, res = _run(inputs, trace=True)
    return out, res
